# revision 20
# baseline (speedup 1.0000x reference)
"""GNN message passing (gnn_message_passing) on 8 Trainium2 NeuronCores.

Computation (see reference):
    out = segment_sum over edges of  w[a] * vals[a,e] * x[src[a,e]]  into rows dst[a,e]
    out = gelu_exact(out / max(||out||_2, 1e-12))   (row-wise L2 normalize)

Strategy (node sharding):
  - Each of the 8 cores owns 6250 destination rows, processed in 49 blocks
    of 128 rows. Host sorts each (core, block)'s incident edges by source
    and cuts them into 4 equal-count runs ("rank quartiles"), packed into
    128-edge tiles (edge p of tile g sits on partition p of the gather
    output).
  - Device, per block: FOUR dma_gather calls (one per rank quartile, on
    SWDGE queues 0-3, each against a per-(block,queue) base view of x so
    offsets stay in int16) pull the block's x[src] rows (fp16) from HBM
    into SBUF as [128 edges, T, 128 feat]. Desc-gen for queue q runs on
    Q7 core pair q; the equal-count split keeps all four pairs busy at
    the measured ~8.4ns/descriptor rate, which is the kernel's wall.
    Indices are 0-padded (pad slots gather x[base]; their one-hot rows
    are zero) so every gx slot is written - no stale-SBUF NaN hazard.
  - The scatter one-hot S0 (0/1 structure, built host-side as uint8) is
    streamed from DRAM and scaled on DVE by vs = w[a]*val[e] (computed
    on device).
  - TensorE accumulates S^T @ X into a PSUM block of 128 output rows.
  - Epilogue pass A (per block): Square+row-accum on ScalarE -> ss, raw
    psum copied to SBUF. Pass B runs in 3 chunks (Sqrt + reciprocal over
    the chunk's columns, then exact GELU with per-partition scale), which
    bounds activation-table reloads and overlaps output DMA with the
    remaining blocks.
  - No collectives - host concatenates the 8 per-core row shards.
"""

import sys

sys.path.insert(0, "/opt/trn_rl_repo")

import os
from contextlib import ExitStack

import numpy as np

import concourse.bass as bass
import concourse.tile as tile
from concourse import bacc, library_config, mybir
from concourse.bass_utils import run_bass_kernel_spmd

N_NODES = 50000
N_HID = 128
N_ADJ = 4
N_EDGE = 600000
N_CORES = 8
RPC = N_NODES // N_CORES          # 6250 destination rows per core
BW = 128                          # destination rows per block
NBLK = (RPC + BW - 1) // BW       # 49 blocks (last block 106 rows)
NQ = 4                            # source quarters == SWDGE queues
QW = (N_NODES + NQ - 1) // NQ     # 12500 source rows per quarter (int16-safe)
EPS = 1e-12

fp16 = mybir.dt.float16
fp32 = mybir.dt.float32
i16 = mybir.dt.int16
u8 = mybir.dt.uint8

LAST_RESULTS = None  # BassKernelResults of the most recent run (for test.py)


def _host_prep(x, weight, adj_src, adj_dst, adj_vals):
    """Partition + sort edges per (core, dst-block, src-quarter); build arrays."""
    x = np.ascontiguousarray(np.asarray(x, dtype=np.float32))
    weight = np.asarray(weight, dtype=np.float32).reshape(N_ADJ)
    src_f = np.asarray(adj_src, dtype=np.int64).reshape(-1)
    dst_f = np.asarray(adj_dst, dtype=np.int64).reshape(-1)
    val_f = np.asarray(adj_vals, dtype=np.float32).reshape(-1)
    aid_f = np.repeat(np.arange(N_ADJ, dtype=np.int64), N_EDGE)

    core = dst_f // RPC
    dloc = dst_f - core * RPC
    blk = dloc // BW                # dst block within core (0..NBLK-1)
    slot = dloc - blk * BW          # dst slot within block (0..BW-1)

    # sort by (core, blk, src) and cut each (core, blk) group into NQ
    # equal-count runs ("rank quartiles") - perfectly balanced SWDGE queues.
    # A quartile of a sorted uniform sample spans ~N_NODES/NQ sources, so
    # per-(blk,q) base views keep offsets well inside int16.
    NG = NBLK * NQ                  # (block, quartile) groups per core
    key_cb = core * NBLK + blk
    order = np.lexsort((src_f, key_cb))
    kcb_s = key_cb[order]
    counts_cb = np.bincount(kcb_s, minlength=N_CORES * NBLK)
    starts_cb = np.zeros(N_CORES * NBLK, dtype=np.int64)
    np.cumsum(counts_cb[:-1], out=starts_cb[1:])
    r0 = np.arange(src_f.size, dtype=np.int64) - np.repeat(starts_cb, counts_cb)
    nb = np.repeat(counts_cb, counts_cb)
    qt_s = (r0 * NQ) // np.maximum(nb, 1)  # 0..NQ-1, sizes within +-1
    ks = kcb_s * NQ + qt_s          # sorted ascending along `order` already

    counts = np.bincount(ks, minlength=N_CORES * NG)
    cnt = counts.reshape(N_CORES, NBLK, NQ)
    # per-(block, quartile) gather size: the max core's true count rounded
    # to the 16-idx wrap granularity. Descriptors are generated only up to
    # this, not the 128-rounded tile count - the last tile's tail slots are
    # never written and rely on the one-time gx memset for finite contents.
    NM16 = ((np.maximum(cnt.max(axis=0), 1) + 15) // 16 * 16).astype(np.int64)
    Tg = np.maximum((NM16 + 127) // 128, 1)          # [NBLK,NQ] tiles
    offs = np.zeros(NBLK * NQ + 1, dtype=np.int64)
    np.cumsum(Tg.reshape(-1), out=offs[1:])
    NT = int(offs[-1])

    # within-group rank of each (sorted) edge
    starts = np.zeros(N_CORES * NG, dtype=np.int64)
    np.cumsum(counts[:-1], out=starts[1:])
    r = np.arange(src_f.size, dtype=np.int64) - np.repeat(starts, counts)
    p = r & 127
    t = r >> 7
    core_s = ks // NG
    grp_s = ks % NG                 # (blk*NQ + qt)
    g = offs[grp_s] + t             # global tile column

    # common (blk, qt) gather base = min over cores of the group's first src
    src_sorted = src_f[order]
    first_src = np.full(N_CORES * NG, N_NODES, dtype=np.int64)
    np.minimum.at(first_src, ks, src_sorted)
    base = first_src.reshape(N_CORES, NG).min(axis=0)   # [NG]
    base[first_src.reshape(N_CORES, NG).min(axis=0) >= N_NODES] = 0
    src_rel_all = src_sorted - base[grp_s]
    assert src_rel_all.min() >= 0 and src_rel_all.max() < 32768, (
        src_rel_all.min(), src_rel_all.max())

    # 0-padded indices: pad slots gather x[quarter_base]; their s0 rows are
    # 0, so every gx slot is written - no stale-SBUF NaN hazard and no
    # per-core count registers needed.
    idx16 = np.zeros((N_CORES, 128, NT), dtype=np.int16)
    v4 = np.zeros((N_CORES, 128, N_ADJ, NT), dtype=np.float16)
    s0 = np.zeros((N_CORES, 128, NT, BW), dtype=np.uint8)  # one-hot rows

    idx16[core_s, p, g] = src_rel_all.astype(np.int16)
    v4[core_s, p, aid_f[order], g] = val_f[order].astype(np.float16)
    s0[core_s, p, g, slot[order]] = 1

    # dma_gather idx layout per group: [16, T*8] wrap (idx j at [j%16, j//16]),
    # replicated to 128 partitions. Build the whole [128, NT*8] slab.
    idxw = np.zeros((N_CORES, 128, NT * 8), dtype=np.int16)
    for gi in range(NG):
        t0, t1 = int(offs[gi]), int(offs[gi + 1])
        n = (t1 - t0) * 128
        flat = idx16[:, :, t0:t1].transpose(0, 2, 1).reshape(N_CORES, n)  # j order
        wrapped = flat.reshape(N_CORES, n // 16, 16).transpose(0, 2, 1)
        idxw[:, :, t0 * 8:t1 * 8] = np.tile(wrapped, (1, 8, 1))

    x16 = x.astype(np.float16)
    return x16, weight, idxw, v4, s0, Tg, offs, NT, base.reshape(NBLK, NQ), NM16


def _build_program(Tg, offs, NT, base, NM16):
    """Build the single-core bass program (same for all 8 cores)."""
    nc = bacc.Bacc("TRN2", target_bir_lowering=False, debug=False,
                   num_swdge_queues=4)

    x_d = nc.dram_tensor("x16", [N_NODES, N_HID], fp16, kind="ExternalInput")
    w_d = nc.dram_tensor("w", [1, N_ADJ], fp32, kind="ExternalInput")
    idx_d = nc.dram_tensor("idxw", [128, NT * 8], i16, kind="ExternalInput")
    v4_d = nc.dram_tensor("v4", [128, N_ADJ * NT], fp16, kind="ExternalInput")
    s0_d = nc.dram_tensor("s0", [128, NT * BW], u8, kind="ExternalInput")
    out_d = nc.dram_tensor("out", [RPC, N_HID], fp32, kind="ExternalOutput")

    AF = mybir.ActivationFunctionType
    OP = mybir.AluOpType

    with tile.TileContext(nc) as tc, ExitStack() as ctx:
        meta = ctx.enter_context(tc.tile_pool(name="meta", bufs=1))

        with tc.high_priority():
            nc.gpsimd.load_library(library_config.mlp)

        # idx slab in 8 chunks (block-aligned) so early gathers start sooner
        idx_sb = meta.tile([128, NT * 8], i16, tag="idx")
        bchunk = (NBLK + 7) // 8
        for ci in range(8):
            b0 = ci * bchunk
            b1 = min(NBLK, b0 + bchunk)
            if b0 >= b1:
                continue
            c0 = int(offs[b0 * NQ]) * 8
            c1 = int(offs[b1 * NQ]) * 8
            nc.sync.dma_start(out=idx_sb[:, c0:c1], in_=idx_d[:, c0:c1])

        vs_sb = meta.tile([128, NT], fp16, tag="vs")
        ss_sb = meta.tile([BW, NBLK], fp32, tag="ss")
        raw_sb = meta.tile([BW, NBLK * N_HID], fp32, tag="raw")
        ssm_sb = meta.tile([BW, NBLK], fp32, tag="ssm")
        nrm_sb = meta.tile([BW, NBLK], fp32, tag="nrm")
        inv_sb = meta.tile([BW, NBLK], fp32, tag="inv")
        rpool = ctx.enter_context(tc.tile_pool(name="res", bufs=4))

        # vs[p, g] = sum_a w[a] * v4[p, a, g]  (fp16); v4/tmps freed after
        with tc.tile_pool(name="v4tmp", bufs=1) as v4pool, \
             tc.tile_pool(name="wtmp", bufs=1, space="PSUM") as wppool:
            v4_sb = v4pool.tile([128, N_ADJ * NT], fp16, tag="v4")
            nc.sync.dma_start(out=v4_sb[:], in_=v4_d[:])
            # broadcast w[4] to 128 partitions via a K=1 matmul with ones
            w1_sb = v4pool.tile([1, N_ADJ], fp32, tag="w1")
            nc.sync.dma_start(out=w1_sb[:], in_=w_d[:])
            ones_sb = v4pool.tile([1, 128], fp32, tag="ones")
            nc.vector.memset(ones_sb[:], 1.0)
            w_ps = wppool.tile([128, N_ADJ], fp32, space="PSUM", tag="wps")
            nc.tensor.matmul(out=w_ps[:], lhsT=ones_sb[:], rhs=w1_sb[:],
                             start=True, stop=True)
            w_bc = v4pool.tile([128, N_ADJ], fp32, tag="wbc")
            nc.vector.tensor_copy(w_bc[:], w_ps[:])

            tmp0 = v4pool.tile([128, NT], fp16, tag="vs_tmp0")
            nc.vector.tensor_scalar(
                out=tmp0[:], in0=v4_sb[:, 0:NT], scalar1=w_bc[:, 0:1],
                scalar2=None, op0=OP.mult)
            tmp1 = v4pool.tile([128, NT], fp16, tag="vs_tmp1")
            nc.vector.scalar_tensor_tensor(
                out=tmp1[:], in0=v4_sb[:, NT:2 * NT], scalar=w_bc[:, 1:2],
                in1=tmp0[:], op0=OP.mult, op1=OP.add)
            nc.vector.scalar_tensor_tensor(
                out=tmp0[:], in0=v4_sb[:, 2 * NT:3 * NT], scalar=w_bc[:, 2:3],
                in1=tmp1[:], op0=OP.mult, op1=OP.add)
            nc.vector.scalar_tensor_tensor(
                out=vs_sb[:], in0=v4_sb[:, 3 * NT:4 * NT], scalar=w_bc[:, 3:4],
                in1=tmp0[:], op0=OP.mult, op1=OP.add)

        gpool = ctx.enter_context(tc.tile_pool(name="gx", bufs=3))
        s0pool = ctx.enter_context(tc.tile_pool(name="s0", bufs=3))
        spool = ctx.enter_context(tc.tile_pool(name="s", bufs=2))
        ppool = ctx.enter_context(tc.tile_pool(name="psum", bufs=4, space="PSUM"))
        epool = ctx.enter_context(tc.tile_pool(name="epi", bufs=2))

        # fixed-shape gx buffers, memset once: slots beyond each gather's
        # num_idxs (the last tile's tail) are never written and must hold
        # finite data so 0 * value matmul products stay 0
        GB = 3
        Tqmax = [int(Tg[:, qq].max()) for qq in range(NQ)]
        for _ in range(GB):
            for qq in range(NQ):
                g = gpool.tile([128, Tqmax[qq], N_HID], fp16, tag=f"gx{qq}")
                nc.vector.memset(g[:], 0.0)

        for b in range(NBLK):
            goff = int(offs[b * NQ])
            Ts = [int(Tg[b, qq]) for qq in range(NQ)]
            nt_b = sum(Ts)
            gxs = []
            for qq in range(NQ):
                T = Ts[qq]
                off_q = int(offs[b * NQ + qq])
                NM = int(NM16[b, qq])
                gx = gpool.tile([128, Tqmax[qq], N_HID], fp16, tag=f"gx{qq}")
                nc.gpsimd.dma_gather(
                    out_ap=gx[:, :T, :], in_ap=x_d[int(base[b, qq]):, :],
                    idxs_ap=idx_sb[:, off_q * 8:(off_q + T) * 8],
                    num_idxs=NM, num_idxs_reg=NM, elem_size=N_HID,
                    single_packet=False, queue_num=qq)
                gxs.append(gx)

            # stream the block's one-hot structure and scale it by vs
            s0_sb = s0pool.tile([128, nt_b, BW], u8, tag="s0")
            nc.sync.dma_start(
                out=s0_sb[:],
                in_=s0_d[:, goff * BW:(goff + nt_b) * BW]
                    .rearrange("p (t f) -> p t f", t=nt_b))
            S = spool.tile([128, nt_b, BW], fp16, tag="S")
            nc.vector.tensor_tensor(
                out=S[:], in0=s0_sb[:],
                in1=vs_sb[:, goff:goff + nt_b].to_broadcast([128, nt_b, BW]),
                op=OP.mult)

            psum = ppool.tile([BW, N_HID], fp32, space="PSUM", tag="acc")
            t = 0
            for qq in range(NQ):
                for tq in range(Ts[qq]):
                    nc.tensor.matmul(
                        out=psum[:], lhsT=S[:, t, :], rhs=gxs[qq][:, tq, :],
                        start=(t == 0), stop=(t == nt_b - 1))
                    t += 1

            # epilogue pass A: row sum-of-squares + stash raw block
            sq = epool.tile([BW, N_HID], fp32, tag="sq")
            nc.scalar.activation(out=sq[:], in_=psum[:], func=AF.Square,
                                 accum_out=ss_sb[:, b:b + 1])
            nc.vector.tensor_copy(raw_sb[:, b * N_HID:(b + 1) * N_HID], psum[:])

            # epilogue pass B in chunks (keeps ACT-table thrash bounded while
            # letting output DMAs overlap the remaining blocks)
            if b in (15, 31, NBLK - 1):
                c0 = {15: 0, 31: 16, NBLK - 1: 32}[b]
                c1 = b + 1
                nc.vector.tensor_scalar(
                    out=ssm_sb[:, c0:c1], in0=ss_sb[:, c0:c1],
                    scalar1=float(EPS * EPS), scalar2=None, op0=OP.max)
                nc.scalar.sqrt(nrm_sb[:, c0:c1], ssm_sb[:, c0:c1])
                nc.vector.reciprocal(inv_sb[:, c0:c1], nrm_sb[:, c0:c1])
                for bb in range(c0, c1):
                    res = rpool.tile([BW, N_HID], fp32, tag="res")
                    nc.scalar.activation(
                        out=res[:],
                        in_=raw_sb[:, bb * N_HID:(bb + 1) * N_HID],
                        func=AF.Gelu, scale=inv_sb[:, bb:bb + 1])
                    rows = min(BW, RPC - bb * BW)
                    nc.sync.dma_start(out=out_d[bb * BW:bb * BW + rows, :],
                                      in_=res[:rows, :])

    nc.compile()
    return nc


def kernel(x, weight, adj_src, adj_dst, adj_vals, _trace=None):
    global LAST_RESULTS
    x16, w, idxw, v4, s0, Tg, offs, NT, base, NM16 = _host_prep(
        x, weight, adj_src, adj_dst, adj_vals)

    nc = _build_program(Tg, offs, NT, base, NM16)

    in_maps = []
    for c in range(N_CORES):
        in_maps.append({
            "x16": x16,
            "w": w.reshape(1, N_ADJ),
            "idxw": idxw[c],
            "v4": v4[c].reshape(128, N_ADJ * NT),
            "s0": s0[c].reshape(128, NT * BW),
        })

    if _trace is None:
        _trace = bool(int(os.environ.get("GNN_TRACE", "0")))
    res = run_bass_kernel_spmd(nc, in_maps, list(range(N_CORES)), trace=_trace)
    LAST_RESULTS = res

    out = np.concatenate([res.results[c]["out"] for c in range(N_CORES)], axis=0)
    return out.astype(np.float32)


# revision 21
# speedup vs baseline: 1.0053x; 1.0053x over previous
"""GNN message passing (gnn_message_passing) on 8 Trainium2 NeuronCores.

Computation (see reference):
    out = segment_sum over edges of  w[a] * vals[a,e] * x[src[a,e]]  into rows dst[a,e]
    out = gelu_exact(out / max(||out||_2, 1e-12))   (row-wise L2 normalize)

Strategy (node sharding):
  - Each of the 8 cores owns 6250 destination rows, processed in 49 blocks
    of 128 rows. Host sorts each (core, block)'s incident edges by source
    and cuts them into 4 equal-count runs ("rank quartiles"), packed into
    128-edge tiles (edge p of tile g sits on partition p of the gather
    output).
  - Device, per block: FOUR dma_gather calls (one per rank quartile, on
    SWDGE queues 0-3, each against a per-(block,queue) base view of x so
    offsets stay in int16) pull the block's x[src] rows (fp16) from HBM
    into SBUF as [128 edges, T, 128 feat]. Desc-gen for queue q runs on
    Q7 core pair q; the equal-count split keeps all four pairs busy at
    the measured ~8.4ns/descriptor rate, which is the kernel's wall.
    Indices are 0-padded (pad slots gather x[base]; their one-hot rows
    are zero) so every gx slot is written - no stale-SBUF NaN hazard.
  - The scatter one-hot S0 (0/1 structure, built host-side as uint8) is
    streamed from DRAM and scaled on DVE by vs = w[a]*val[e] (computed
    on device).
  - TensorE accumulates S^T @ X into a PSUM block of 128 output rows.
  - Epilogue pass A (per block): Square+row-accum on ScalarE -> ss, raw
    psum copied to SBUF. Pass B runs in 3 chunks (Sqrt + reciprocal over
    the chunk's columns, then exact GELU with per-partition scale), which
    bounds activation-table reloads and overlaps output DMA with the
    remaining blocks.
  - No collectives - host concatenates the 8 per-core row shards.
"""

import sys

sys.path.insert(0, "/opt/trn_rl_repo")

import os
from contextlib import ExitStack

import numpy as np

import concourse.bass as bass
import concourse.tile as tile
from concourse import bacc, library_config, mybir
from concourse.bass_utils import run_bass_kernel_spmd

N_NODES = 50000
N_HID = 128
N_ADJ = 4
N_EDGE = 600000
N_CORES = 8
RPC = N_NODES // N_CORES          # 6250 destination rows per core
BW = 128                          # destination rows per block
NBLK = (RPC + BW - 1) // BW       # 49 blocks (last block 106 rows)
NQ = 4                            # source quarters == SWDGE queues
QW = (N_NODES + NQ - 1) // NQ     # 12500 source rows per quarter (int16-safe)
EPS = 1e-12

fp16 = mybir.dt.float16
fp32 = mybir.dt.float32
i16 = mybir.dt.int16
u8 = mybir.dt.uint8

LAST_RESULTS = None  # BassKernelResults of the most recent run (for test.py)


def _host_prep(x, weight, adj_src, adj_dst, adj_vals):
    """Partition + sort edges per (core, dst-block, src-quarter); build arrays."""
    x = np.ascontiguousarray(np.asarray(x, dtype=np.float32))
    weight = np.asarray(weight, dtype=np.float32).reshape(N_ADJ)
    src_f = np.asarray(adj_src, dtype=np.int64).reshape(-1)
    dst_f = np.asarray(adj_dst, dtype=np.int64).reshape(-1)
    val_f = np.asarray(adj_vals, dtype=np.float32).reshape(-1)
    aid_f = np.repeat(np.arange(N_ADJ, dtype=np.int64), N_EDGE)

    core = dst_f // RPC
    dloc = dst_f - core * RPC
    blk = dloc // BW                # dst block within core (0..NBLK-1)
    slot = dloc - blk * BW          # dst slot within block (0..BW-1)

    # sort by (core, blk, src) and cut each (core, blk) group into NQ
    # equal-count runs ("rank quartiles") - perfectly balanced SWDGE queues.
    # A quartile of a sorted uniform sample spans ~N_NODES/NQ sources, so
    # per-(blk,q) base views keep offsets well inside int16.
    NG = NBLK * NQ                  # (block, quartile) groups per core
    key_cb = core * NBLK + blk
    order = np.lexsort((src_f, key_cb))
    kcb_s = key_cb[order]
    counts_cb = np.bincount(kcb_s, minlength=N_CORES * NBLK)
    starts_cb = np.zeros(N_CORES * NBLK, dtype=np.int64)
    np.cumsum(counts_cb[:-1], out=starts_cb[1:])
    r0 = np.arange(src_f.size, dtype=np.int64) - np.repeat(starts_cb, counts_cb)
    nb = np.repeat(counts_cb, counts_cb)
    qt_s = (r0 * NQ) // np.maximum(nb, 1)  # 0..NQ-1, sizes within +-1
    ks = kcb_s * NQ + qt_s          # sorted ascending along `order` already

    counts = np.bincount(ks, minlength=N_CORES * NG)
    cnt = counts.reshape(N_CORES, NBLK, NQ)
    # per-(block, quartile) gather size: the max core's true count rounded
    # to the 16-idx wrap granularity. Descriptors are generated only up to
    # this, not the 128-rounded tile count - the last tile's tail slots are
    # never written and rely on the one-time gx memset for finite contents.
    NM16 = ((np.maximum(cnt.max(axis=0), 1) + 15) // 16 * 16).astype(np.int64)
    Tg = np.maximum((NM16 + 127) // 128, 1)          # [NBLK,NQ] tiles
    offs = np.zeros(NBLK * NQ + 1, dtype=np.int64)
    np.cumsum(Tg.reshape(-1), out=offs[1:])
    NT = int(offs[-1])

    # within-group rank of each (sorted) edge
    starts = np.zeros(N_CORES * NG, dtype=np.int64)
    np.cumsum(counts[:-1], out=starts[1:])
    r = np.arange(src_f.size, dtype=np.int64) - np.repeat(starts, counts)
    p = r & 127
    t = r >> 7
    core_s = ks // NG
    grp_s = ks % NG                 # (blk*NQ + qt)
    g = offs[grp_s] + t             # global tile column

    # common (blk, qt) gather base = min over cores of the group's first src
    src_sorted = src_f[order]
    first_src = np.full(N_CORES * NG, N_NODES, dtype=np.int64)
    np.minimum.at(first_src, ks, src_sorted)
    base = first_src.reshape(N_CORES, NG).min(axis=0)   # [NG]
    base[first_src.reshape(N_CORES, NG).min(axis=0) >= N_NODES] = 0
    src_rel_all = src_sorted - base[grp_s]
    assert src_rel_all.min() >= 0 and src_rel_all.max() < 32768, (
        src_rel_all.min(), src_rel_all.max())

    # 0-padded indices: pad slots gather x[quarter_base]; their s0 rows are
    # 0, so every gx slot is written - no stale-SBUF NaN hazard and no
    # per-core count registers needed.
    idx16 = np.zeros((N_CORES, 128, NT), dtype=np.int16)
    v4 = np.zeros((N_CORES, 128, N_ADJ, NT), dtype=np.float16)
    s0 = np.zeros((N_CORES, 128, NT, BW), dtype=np.uint8)  # one-hot rows

    idx16[core_s, p, g] = src_rel_all.astype(np.int16)
    v4[core_s, p, aid_f[order], g] = val_f[order].astype(np.float16)
    s0[core_s, p, g, slot[order]] = 1

    # dma_gather idx layout per group: [16, T*8] wrap (idx j at [j%16, j//16]),
    # replicated to 128 partitions. Build the whole [128, NT*8] slab.
    idxw = np.zeros((N_CORES, 128, NT * 8), dtype=np.int16)
    for gi in range(NG):
        t0, t1 = int(offs[gi]), int(offs[gi + 1])
        n = (t1 - t0) * 128
        flat = idx16[:, :, t0:t1].transpose(0, 2, 1).reshape(N_CORES, n)  # j order
        wrapped = flat.reshape(N_CORES, n // 16, 16).transpose(0, 2, 1)
        idxw[:, :, t0 * 8:t1 * 8] = np.tile(wrapped, (1, 8, 1))

    x16 = x.astype(np.float16)
    return x16, weight, idxw, v4, s0, Tg, offs, NT, base.reshape(NBLK, NQ), NM16


def _build_program(Tg, offs, NT, base, NM16):
    """Build the single-core bass program (same for all 8 cores)."""
    nc = bacc.Bacc("TRN2", target_bir_lowering=False, debug=False,
                   num_swdge_queues=4)

    x_d = nc.dram_tensor("x16", [N_NODES, N_HID], fp16, kind="ExternalInput")
    w_d = nc.dram_tensor("w", [1, N_ADJ], fp32, kind="ExternalInput")
    idx_d = nc.dram_tensor("idxw", [128, NT * 8], i16, kind="ExternalInput")
    v4_d = nc.dram_tensor("v4", [128, N_ADJ * NT], fp16, kind="ExternalInput")
    s0_d = nc.dram_tensor("s0", [128, NT * BW], u8, kind="ExternalInput")
    out_d = nc.dram_tensor("out", [RPC, N_HID], fp32, kind="ExternalOutput")

    AF = mybir.ActivationFunctionType
    OP = mybir.AluOpType

    with tile.TileContext(nc) as tc, ExitStack() as ctx:
        meta = ctx.enter_context(tc.tile_pool(name="meta", bufs=1))

        with tc.high_priority():
            nc.gpsimd.load_library(library_config.mlp)

        # idx slab in 8 chunks (block-aligned) so early gathers start sooner
        idx_sb = meta.tile([128, NT * 8], i16, tag="idx")
        bchunk = (NBLK + 7) // 8
        for ci in range(8):
            b0 = ci * bchunk
            b1 = min(NBLK, b0 + bchunk)
            if b0 >= b1:
                continue
            c0 = int(offs[b0 * NQ]) * 8
            c1 = int(offs[b1 * NQ]) * 8
            nc.sync.dma_start(out=idx_sb[:, c0:c1], in_=idx_d[:, c0:c1])

        vs_sb = meta.tile([128, NT], fp16, tag="vs")
        ss_sb = meta.tile([BW, NBLK], fp32, tag="ss")
        raw_sb = meta.tile([BW, NBLK * N_HID], fp32, tag="raw")
        ssm_sb = meta.tile([BW, NBLK], fp32, tag="ssm")
        nrm_sb = meta.tile([BW, NBLK], fp32, tag="nrm")
        inv_sb = meta.tile([BW, NBLK], fp32, tag="inv")
        rpool = ctx.enter_context(tc.tile_pool(name="res", bufs=4))

        # vs[p, g] = sum_a w[a] * v4[p, a, g]  (fp16); v4/tmps freed after
        with tc.tile_pool(name="v4tmp", bufs=1) as v4pool, \
             tc.tile_pool(name="wtmp", bufs=1, space="PSUM") as wppool:
            v4_sb = v4pool.tile([128, N_ADJ * NT], fp16, tag="v4")
            nc.sync.dma_start(out=v4_sb[:], in_=v4_d[:])
            # broadcast w[4] to 128 partitions via a K=1 matmul with ones
            w1_sb = v4pool.tile([1, N_ADJ], fp32, tag="w1")
            nc.sync.dma_start(out=w1_sb[:], in_=w_d[:])
            ones_sb = v4pool.tile([1, 128], fp32, tag="ones")
            nc.vector.memset(ones_sb[:], 1.0)
            w_ps = wppool.tile([128, N_ADJ], fp32, space="PSUM", tag="wps")
            nc.tensor.matmul(out=w_ps[:], lhsT=ones_sb[:], rhs=w1_sb[:],
                             start=True, stop=True)
            w_bc = v4pool.tile([128, N_ADJ], fp32, tag="wbc")
            nc.vector.tensor_copy(w_bc[:], w_ps[:])

            tmp0 = v4pool.tile([128, NT], fp16, tag="vs_tmp0")
            nc.vector.tensor_scalar(
                out=tmp0[:], in0=v4_sb[:, 0:NT], scalar1=w_bc[:, 0:1],
                scalar2=None, op0=OP.mult)
            tmp1 = v4pool.tile([128, NT], fp16, tag="vs_tmp1")
            nc.vector.scalar_tensor_tensor(
                out=tmp1[:], in0=v4_sb[:, NT:2 * NT], scalar=w_bc[:, 1:2],
                in1=tmp0[:], op0=OP.mult, op1=OP.add)
            nc.vector.scalar_tensor_tensor(
                out=tmp0[:], in0=v4_sb[:, 2 * NT:3 * NT], scalar=w_bc[:, 2:3],
                in1=tmp1[:], op0=OP.mult, op1=OP.add)
            nc.vector.scalar_tensor_tensor(
                out=vs_sb[:], in0=v4_sb[:, 3 * NT:4 * NT], scalar=w_bc[:, 3:4],
                in1=tmp0[:], op0=OP.mult, op1=OP.add)

        gpool = ctx.enter_context(tc.tile_pool(name="gx", bufs=4))
        s0pool = ctx.enter_context(tc.tile_pool(name="s0", bufs=4))
        spool = ctx.enter_context(tc.tile_pool(name="s", bufs=2))
        ppool = ctx.enter_context(tc.tile_pool(name="psum", bufs=4, space="PSUM"))
        epool = ctx.enter_context(tc.tile_pool(name="epi", bufs=2))

        # fixed-shape gx buffers, memset once: slots beyond each gather's
        # num_idxs (the last tile's tail) are never written and must hold
        # finite data so 0 * value matmul products stay 0
        GB = 4
        Tqmax = [int(Tg[:, qq].max()) for qq in range(NQ)]
        for _ in range(GB):
            for qq in range(NQ):
                g = gpool.tile([128, Tqmax[qq], N_HID], fp16, tag=f"gx{qq}")
                nc.vector.memset(g[:], 0.0)

        for b in range(NBLK):
            goff = int(offs[b * NQ])
            Ts = [int(Tg[b, qq]) for qq in range(NQ)]
            nt_b = sum(Ts)
            gxs = []
            for qq in range(NQ):
                T = Ts[qq]
                off_q = int(offs[b * NQ + qq])
                NM = int(NM16[b, qq])
                gx = gpool.tile([128, Tqmax[qq], N_HID], fp16, tag=f"gx{qq}")
                nc.gpsimd.dma_gather(
                    out_ap=gx[:, :T, :], in_ap=x_d[int(base[b, qq]):, :],
                    idxs_ap=idx_sb[:, off_q * 8:(off_q + T) * 8],
                    num_idxs=NM, num_idxs_reg=NM, elem_size=N_HID,
                    single_packet=False, queue_num=qq)
                gxs.append(gx)

            # stream the block's one-hot structure and scale it by vs
            s0_sb = s0pool.tile([128, nt_b, BW], u8, tag="s0")
            nc.sync.dma_start(
                out=s0_sb[:],
                in_=s0_d[:, goff * BW:(goff + nt_b) * BW]
                    .rearrange("p (t f) -> p t f", t=nt_b))
            S = spool.tile([128, nt_b, BW], fp16, tag="S")
            nc.vector.tensor_tensor(
                out=S[:], in0=s0_sb[:],
                in1=vs_sb[:, goff:goff + nt_b].to_broadcast([128, nt_b, BW]),
                op=OP.mult)

            psum = ppool.tile([BW, N_HID], fp32, space="PSUM", tag="acc")
            t = 0
            for qq in range(NQ):
                for tq in range(Ts[qq]):
                    nc.tensor.matmul(
                        out=psum[:], lhsT=S[:, t, :], rhs=gxs[qq][:, tq, :],
                        start=(t == 0), stop=(t == nt_b - 1))
                    t += 1

            # epilogue pass A: row sum-of-squares + stash raw block
            sq = epool.tile([BW, N_HID], fp32, tag="sq")
            nc.scalar.activation(out=sq[:], in_=psum[:], func=AF.Square,
                                 accum_out=ss_sb[:, b:b + 1])
            nc.vector.tensor_copy(raw_sb[:, b * N_HID:(b + 1) * N_HID], psum[:])

            # epilogue pass B in chunks (keeps ACT-table thrash bounded while
            # letting output DMAs overlap the remaining blocks)
            if b in (15, 31, NBLK - 1):
                c0 = {15: 0, 31: 16, NBLK - 1: 32}[b]
                c1 = b + 1
                nc.vector.tensor_scalar(
                    out=ssm_sb[:, c0:c1], in0=ss_sb[:, c0:c1],
                    scalar1=float(EPS * EPS), scalar2=None, op0=OP.max)
                nc.scalar.sqrt(nrm_sb[:, c0:c1], ssm_sb[:, c0:c1])
                nc.vector.reciprocal(inv_sb[:, c0:c1], nrm_sb[:, c0:c1])
                for bb in range(c0, c1):
                    res = rpool.tile([BW, N_HID], fp32, tag="res")
                    nc.scalar.activation(
                        out=res[:],
                        in_=raw_sb[:, bb * N_HID:(bb + 1) * N_HID],
                        func=AF.Gelu, scale=inv_sb[:, bb:bb + 1])
                    rows = min(BW, RPC - bb * BW)
                    nc.sync.dma_start(out=out_d[bb * BW:bb * BW + rows, :],
                                      in_=res[:rows, :])

    nc.compile()
    return nc


def kernel(x, weight, adj_src, adj_dst, adj_vals, _trace=None):
    global LAST_RESULTS
    x16, w, idxw, v4, s0, Tg, offs, NT, base, NM16 = _host_prep(
        x, weight, adj_src, adj_dst, adj_vals)

    nc = _build_program(Tg, offs, NT, base, NM16)

    in_maps = []
    for c in range(N_CORES):
        in_maps.append({
            "x16": x16,
            "w": w.reshape(1, N_ADJ),
            "idxw": idxw[c],
            "v4": v4[c].reshape(128, N_ADJ * NT),
            "s0": s0[c].reshape(128, NT * BW),
        })

    if _trace is None:
        _trace = bool(int(os.environ.get("GNN_TRACE", "0")))
    res = run_bass_kernel_spmd(nc, in_maps, list(range(N_CORES)), trace=_trace)
    LAST_RESULTS = res

    out = np.concatenate([res.results[c]["out"] for c in range(N_CORES)], axis=0)
    return out.astype(np.float32)


# revision 22
# speedup vs baseline: 1.0288x; 1.0234x over previous
"""GNN message passing (gnn_message_passing) on 8 Trainium2 NeuronCores.

Computation (see reference):
    out = segment_sum over edges of  w[a] * vals[a,e] * x[src[a,e]]  into rows dst[a,e]
    out = gelu_exact(out / max(||out||_2, 1e-12))   (row-wise L2 normalize)

Strategy (node sharding):
  - Each of the 8 cores owns 6250 destination rows, processed in 49 blocks
    of 128 rows. Host sorts each (core, block)'s incident edges by source
    and cuts them into 4 equal-count runs ("rank quartiles"), packed into
    128-edge tiles (edge p of tile g sits on partition p of the gather
    output).
  - Device, per block: FOUR dma_gather calls (one per rank quartile, on
    SWDGE queues 0-3, each against a per-(block,queue) base view of x so
    offsets stay in int16) pull the block's x[src] rows (fp16) from HBM
    into SBUF as [128 edges, T, 128 feat]. Desc-gen for queue q runs on
    Q7 core pair q; the equal-count split keeps all four pairs busy at
    the measured ~8.4ns/descriptor rate, which is the kernel's wall.
    Indices are 0-padded (pad slots gather x[base]; their one-hot rows
    are zero) so every gx slot is written - no stale-SBUF NaN hazard.
  - The scatter one-hot S0 (0/1 structure, built host-side as uint8) is
    streamed from DRAM and scaled on DVE by vs = w[a]*val[e] (computed
    on device).
  - TensorE accumulates S^T @ X into a PSUM block of 128 output rows.
  - Epilogue pass A (per block): Square+row-accum on ScalarE -> ss, raw
    psum copied to SBUF. Pass B runs in 3 chunks (Sqrt + reciprocal over
    the chunk's columns, then exact GELU with per-partition scale), which
    bounds activation-table reloads and overlaps output DMA with the
    remaining blocks.
  - No collectives - host concatenates the 8 per-core row shards.
"""

import sys

sys.path.insert(0, "/opt/trn_rl_repo")

import os
from contextlib import ExitStack

import numpy as np

import concourse.bass as bass
import concourse.tile as tile
from concourse import bacc, library_config, mybir
from concourse.bass_utils import run_bass_kernel_spmd

N_NODES = 50000
N_HID = 128
N_ADJ = 4
N_EDGE = 600000
N_CORES = 8
RPC = N_NODES // N_CORES          # 6250 destination rows per core
BW = 128                          # destination rows per block
NBLK = (RPC + BW - 1) // BW       # 49 blocks (last block 106 rows)
NQ = 4                            # source quarters == SWDGE queues
QW = (N_NODES + NQ - 1) // NQ     # 12500 source rows per quarter (int16-safe)
EPS = 1e-12

fp16 = mybir.dt.float16
fp32 = mybir.dt.float32
i16 = mybir.dt.int16
u8 = mybir.dt.uint8

LAST_RESULTS = None  # BassKernelResults of the most recent run (for test.py)


def _host_prep(x, weight, adj_src, adj_dst, adj_vals):
    """Partition + sort edges per (core, dst-block, src-quarter); build arrays."""
    x = np.ascontiguousarray(np.asarray(x, dtype=np.float32))
    weight = np.asarray(weight, dtype=np.float32).reshape(N_ADJ)
    src_f = np.asarray(adj_src, dtype=np.int64).reshape(-1)
    dst_f = np.asarray(adj_dst, dtype=np.int64).reshape(-1)
    val_f = np.asarray(adj_vals, dtype=np.float32).reshape(-1)
    aid_f = np.repeat(np.arange(N_ADJ, dtype=np.int64), N_EDGE)

    core = dst_f // RPC
    dloc = dst_f - core * RPC
    blk = dloc // BW                # dst block within core (0..NBLK-1)
    slot = dloc - blk * BW          # dst slot within block (0..BW-1)

    # sort by (core, blk, src) and cut each (core, blk) group into NQ
    # equal-count runs ("rank quartiles") - perfectly balanced SWDGE queues.
    # A quartile of a sorted uniform sample spans ~N_NODES/NQ sources, so
    # per-(blk,q) base views keep offsets well inside int16.
    NG = NBLK * NQ                  # (block, quartile) groups per core
    key_cb = core * NBLK + blk
    order = np.lexsort((src_f, key_cb))
    kcb_s = key_cb[order]
    counts_cb = np.bincount(kcb_s, minlength=N_CORES * NBLK)
    starts_cb = np.zeros(N_CORES * NBLK, dtype=np.int64)
    np.cumsum(counts_cb[:-1], out=starts_cb[1:])
    r0 = np.arange(src_f.size, dtype=np.int64) - np.repeat(starts_cb, counts_cb)
    # capacity-quantized cut: desc-gen works in 128-idx chunks, so cut each
    # (core, blk) run at 128-aligned capacities totalling the minimum chunk
    # count ceil(nb_max/128). The odd chunks rotate across queues by block
    # (issue order unchanged) so the four Q7 pairs carry equal long-run
    # loads and the gx/s0 buffer depth smooths the per-block +-1 imbalance.
    nb_max = counts_cb.reshape(N_CORES, NBLK).max(axis=0)         # [NBLK]
    Ttot = np.maximum((nb_max + 127) // 128, NQ)                  # chunks/blk
    sizes = Ttot[:, None] // NQ + (np.arange(NQ)[None, :] < (Ttot[:, None] % NQ))
    rot = (np.arange(NQ)[None, :] + np.arange(NBLK)[:, None]) % NQ
    caps = 128 * np.take_along_axis(sizes, rot, axis=1)           # [NBLK, NQ]
    cum = np.cumsum(caps, axis=1)                                 # [NBLK, NQ]
    blk_s = kcb_s % NBLK
    qt_s = (r0[:, None] >= cum[blk_s][:, :NQ - 1]).sum(axis=1)
    ks = kcb_s * NQ + qt_s          # sorted ascending along `order` already

    counts = np.bincount(ks, minlength=N_CORES * NG)
    cnt = counts.reshape(N_CORES, NBLK, NQ)
    # per-(block, quartile) gather size: the max core's true count rounded
    # to the 16-idx wrap granularity. Descriptors are generated only up to
    # this, not the 128-rounded tile count - the last tile's tail slots are
    # never written and rely on the one-time gx memset for finite contents.
    NM16 = ((np.maximum(cnt.max(axis=0), 1) + 15) // 16 * 16).astype(np.int64)
    Tg = np.maximum((NM16 + 127) // 128, 1)          # [NBLK,NQ] tiles
    offs = np.zeros(NBLK * NQ + 1, dtype=np.int64)
    np.cumsum(Tg.reshape(-1), out=offs[1:])
    NT = int(offs[-1])

    # within-group rank of each (sorted) edge
    starts = np.zeros(N_CORES * NG, dtype=np.int64)
    np.cumsum(counts[:-1], out=starts[1:])
    r = np.arange(src_f.size, dtype=np.int64) - np.repeat(starts, counts)
    p = r & 127
    t = r >> 7
    core_s = ks // NG
    grp_s = ks % NG                 # (blk*NQ + qt)
    g = offs[grp_s] + t             # global tile column

    # common (blk, qt) gather base = min over cores of the group's first src
    src_sorted = src_f[order]
    first_src = np.full(N_CORES * NG, N_NODES, dtype=np.int64)
    np.minimum.at(first_src, ks, src_sorted)
    base = first_src.reshape(N_CORES, NG).min(axis=0)   # [NG]
    base[first_src.reshape(N_CORES, NG).min(axis=0) >= N_NODES] = 0
    src_rel_all = src_sorted - base[grp_s]
    assert src_rel_all.min() >= 0 and src_rel_all.max() < 32768, (
        src_rel_all.min(), src_rel_all.max())

    # 0-padded indices: pad slots gather x[quarter_base]; their s0 rows are
    # 0, so every gx slot is written - no stale-SBUF NaN hazard and no
    # per-core count registers needed.
    idx16 = np.zeros((N_CORES, 128, NT), dtype=np.int16)
    v4 = np.zeros((N_CORES, 128, N_ADJ, NT), dtype=np.float16)
    s0 = np.zeros((N_CORES, 128, NT, BW), dtype=np.uint8)  # one-hot rows

    idx16[core_s, p, g] = src_rel_all.astype(np.int16)
    v4[core_s, p, aid_f[order], g] = val_f[order].astype(np.float16)
    s0[core_s, p, g, slot[order]] = 1

    # dma_gather idx layout per group: [16, T*8] wrap (idx j at [j%16, j//16]),
    # replicated to 128 partitions. Build the whole [128, NT*8] slab.
    idxw = np.zeros((N_CORES, 128, NT * 8), dtype=np.int16)
    for gi in range(NG):
        t0, t1 = int(offs[gi]), int(offs[gi + 1])
        n = (t1 - t0) * 128
        flat = idx16[:, :, t0:t1].transpose(0, 2, 1).reshape(N_CORES, n)  # j order
        wrapped = flat.reshape(N_CORES, n // 16, 16).transpose(0, 2, 1)
        idxw[:, :, t0 * 8:t1 * 8] = np.tile(wrapped, (1, 8, 1))

    x16 = x.astype(np.float16)
    return x16, weight, idxw, v4, s0, Tg, offs, NT, base.reshape(NBLK, NQ), NM16


def _build_program(Tg, offs, NT, base, NM16):
    """Build the single-core bass program (same for all 8 cores)."""
    nc = bacc.Bacc("TRN2", target_bir_lowering=False, debug=False,
                   num_swdge_queues=4)

    x_d = nc.dram_tensor("x16", [N_NODES, N_HID], fp16, kind="ExternalInput")
    w_d = nc.dram_tensor("w", [1, N_ADJ], fp32, kind="ExternalInput")
    idx_d = nc.dram_tensor("idxw", [128, NT * 8], i16, kind="ExternalInput")
    v4_d = nc.dram_tensor("v4", [128, N_ADJ * NT], fp16, kind="ExternalInput")
    s0_d = nc.dram_tensor("s0", [128, NT * BW], u8, kind="ExternalInput")
    out_d = nc.dram_tensor("out", [RPC, N_HID], fp32, kind="ExternalOutput")

    AF = mybir.ActivationFunctionType
    OP = mybir.AluOpType

    with tile.TileContext(nc) as tc, ExitStack() as ctx:
        meta = ctx.enter_context(tc.tile_pool(name="meta", bufs=1))

        with tc.high_priority():
            nc.gpsimd.load_library(library_config.mlp)

        # idx slab in 8 chunks (block-aligned) so early gathers start sooner
        idx_sb = meta.tile([128, NT * 8], i16, tag="idx")
        bchunk = (NBLK + 7) // 8
        for ci in range(8):
            b0 = ci * bchunk
            b1 = min(NBLK, b0 + bchunk)
            if b0 >= b1:
                continue
            c0 = int(offs[b0 * NQ]) * 8
            c1 = int(offs[b1 * NQ]) * 8
            nc.sync.dma_start(out=idx_sb[:, c0:c1], in_=idx_d[:, c0:c1])

        vs_sb = meta.tile([128, NT], fp16, tag="vs")
        ss_sb = meta.tile([BW, NBLK], fp32, tag="ss")
        raw_sb = meta.tile([BW, NBLK * N_HID], fp32, tag="raw")
        ssm_sb = meta.tile([BW, NBLK], fp32, tag="ssm")
        nrm_sb = meta.tile([BW, NBLK], fp32, tag="nrm")
        inv_sb = meta.tile([BW, NBLK], fp32, tag="inv")
        rpool = ctx.enter_context(tc.tile_pool(name="res", bufs=4))

        # vs[p, g] = sum_a w[a] * v4[p, a, g]  (fp16); v4/tmps freed after
        with tc.tile_pool(name="v4tmp", bufs=1) as v4pool, \
             tc.tile_pool(name="wtmp", bufs=1, space="PSUM") as wppool:
            v4_sb = v4pool.tile([128, N_ADJ * NT], fp16, tag="v4")
            nc.sync.dma_start(out=v4_sb[:], in_=v4_d[:])
            # broadcast w[4] to 128 partitions via a K=1 matmul with ones
            w1_sb = v4pool.tile([1, N_ADJ], fp32, tag="w1")
            nc.sync.dma_start(out=w1_sb[:], in_=w_d[:])
            ones_sb = v4pool.tile([1, 128], fp32, tag="ones")
            nc.vector.memset(ones_sb[:], 1.0)
            w_ps = wppool.tile([128, N_ADJ], fp32, space="PSUM", tag="wps")
            nc.tensor.matmul(out=w_ps[:], lhsT=ones_sb[:], rhs=w1_sb[:],
                             start=True, stop=True)
            w_bc = v4pool.tile([128, N_ADJ], fp32, tag="wbc")
            nc.vector.tensor_copy(w_bc[:], w_ps[:])

            tmp0 = v4pool.tile([128, NT], fp16, tag="vs_tmp0")
            nc.vector.tensor_scalar(
                out=tmp0[:], in0=v4_sb[:, 0:NT], scalar1=w_bc[:, 0:1],
                scalar2=None, op0=OP.mult)
            tmp1 = v4pool.tile([128, NT], fp16, tag="vs_tmp1")
            nc.vector.scalar_tensor_tensor(
                out=tmp1[:], in0=v4_sb[:, NT:2 * NT], scalar=w_bc[:, 1:2],
                in1=tmp0[:], op0=OP.mult, op1=OP.add)
            nc.vector.scalar_tensor_tensor(
                out=tmp0[:], in0=v4_sb[:, 2 * NT:3 * NT], scalar=w_bc[:, 2:3],
                in1=tmp1[:], op0=OP.mult, op1=OP.add)
            nc.vector.scalar_tensor_tensor(
                out=vs_sb[:], in0=v4_sb[:, 3 * NT:4 * NT], scalar=w_bc[:, 3:4],
                in1=tmp0[:], op0=OP.mult, op1=OP.add)

        gpool = ctx.enter_context(tc.tile_pool(name="gx", bufs=4))
        s0pool = ctx.enter_context(tc.tile_pool(name="s0", bufs=4))
        spool = ctx.enter_context(tc.tile_pool(name="s", bufs=2))
        ppool = ctx.enter_context(tc.tile_pool(name="psum", bufs=4, space="PSUM"))
        epool = ctx.enter_context(tc.tile_pool(name="epi", bufs=2))

        # fixed-shape gx buffers, memset once: slots beyond each gather's
        # num_idxs (the last tile's tail) are never written and must hold
        # finite data so 0 * value matmul products stay 0
        GB = 4
        Tqmax = [int(Tg[:, qq].max()) for qq in range(NQ)]
        for _ in range(GB):
            for qq in range(NQ):
                g = gpool.tile([128, Tqmax[qq], N_HID], fp16, tag=f"gx{qq}")
                nc.vector.memset(g[:], 0.0)

        for b in range(NBLK):
            goff = int(offs[b * NQ])
            Ts = [int(Tg[b, qq]) for qq in range(NQ)]
            nt_b = sum(Ts)
            gxs = []
            for qq in range(NQ):
                T = Ts[qq]
                off_q = int(offs[b * NQ + qq])
                NM = int(NM16[b, qq])
                gx = gpool.tile([128, Tqmax[qq], N_HID], fp16, tag=f"gx{qq}")
                nc.gpsimd.dma_gather(
                    out_ap=gx[:, :T, :], in_ap=x_d[int(base[b, qq]):, :],
                    idxs_ap=idx_sb[:, off_q * 8:(off_q + T) * 8],
                    num_idxs=NM, num_idxs_reg=NM, elem_size=N_HID,
                    single_packet=False, queue_num=qq)
                gxs.append(gx)

            # stream the block's one-hot structure and scale it by vs
            s0_sb = s0pool.tile([128, nt_b, BW], u8, tag="s0")
            nc.sync.dma_start(
                out=s0_sb[:],
                in_=s0_d[:, goff * BW:(goff + nt_b) * BW]
                    .rearrange("p (t f) -> p t f", t=nt_b))
            S = spool.tile([128, nt_b, BW], fp16, tag="S")
            nc.vector.tensor_tensor(
                out=S[:], in0=s0_sb[:],
                in1=vs_sb[:, goff:goff + nt_b].to_broadcast([128, nt_b, BW]),
                op=OP.mult)

            psum = ppool.tile([BW, N_HID], fp32, space="PSUM", tag="acc")
            t = 0
            for qq in range(NQ):
                for tq in range(Ts[qq]):
                    nc.tensor.matmul(
                        out=psum[:], lhsT=S[:, t, :], rhs=gxs[qq][:, tq, :],
                        start=(t == 0), stop=(t == nt_b - 1))
                    t += 1

            # epilogue pass A: row sum-of-squares + stash raw block
            sq = epool.tile([BW, N_HID], fp32, tag="sq")
            nc.scalar.activation(out=sq[:], in_=psum[:], func=AF.Square,
                                 accum_out=ss_sb[:, b:b + 1])
            nc.vector.tensor_copy(raw_sb[:, b * N_HID:(b + 1) * N_HID], psum[:])

            # epilogue pass B in chunks (keeps ACT-table thrash bounded while
            # letting output DMAs overlap the remaining blocks)
            if b in (15, 31, NBLK - 1):
                c0 = {15: 0, 31: 16, NBLK - 1: 32}[b]
                c1 = b + 1
                nc.vector.tensor_scalar(
                    out=ssm_sb[:, c0:c1], in0=ss_sb[:, c0:c1],
                    scalar1=float(EPS * EPS), scalar2=None, op0=OP.max)
                nc.scalar.sqrt(nrm_sb[:, c0:c1], ssm_sb[:, c0:c1])
                nc.vector.reciprocal(inv_sb[:, c0:c1], nrm_sb[:, c0:c1])
                for bb in range(c0, c1):
                    res = rpool.tile([BW, N_HID], fp32, tag="res")
                    nc.scalar.activation(
                        out=res[:],
                        in_=raw_sb[:, bb * N_HID:(bb + 1) * N_HID],
                        func=AF.Gelu, scale=inv_sb[:, bb:bb + 1])
                    rows = min(BW, RPC - bb * BW)
                    nc.sync.dma_start(out=out_d[bb * BW:bb * BW + rows, :],
                                      in_=res[:rows, :])

    nc.compile()
    return nc


def kernel(x, weight, adj_src, adj_dst, adj_vals, _trace=None):
    global LAST_RESULTS
    x16, w, idxw, v4, s0, Tg, offs, NT, base, NM16 = _host_prep(
        x, weight, adj_src, adj_dst, adj_vals)

    nc = _build_program(Tg, offs, NT, base, NM16)

    in_maps = []
    for c in range(N_CORES):
        in_maps.append({
            "x16": x16,
            "w": w.reshape(1, N_ADJ),
            "idxw": idxw[c],
            "v4": v4[c].reshape(128, N_ADJ * NT),
            "s0": s0[c].reshape(128, NT * BW),
        })

    if _trace is None:
        _trace = bool(int(os.environ.get("GNN_TRACE", "0")))
    res = run_bass_kernel_spmd(nc, in_maps, list(range(N_CORES)), trace=_trace)
    LAST_RESULTS = res

    out = np.concatenate([res.results[c]["out"] for c in range(N_CORES)], axis=0)
    return out.astype(np.float32)


# revision 23
# speedup vs baseline: 1.0328x; 1.0039x over previous
"""GNN message passing (gnn_message_passing) on 8 Trainium2 NeuronCores.

Computation (see reference):
    out = segment_sum over edges of  w[a] * vals[a,e] * x[src[a,e]]  into rows dst[a,e]
    out = gelu_exact(out / max(||out||_2, 1e-12))   (row-wise L2 normalize)

Strategy (node sharding):
  - Each of the 8 cores owns 6250 destination rows, processed in 49 blocks
    of 128 rows. Host sorts each (core, block)'s incident edges by source
    and cuts them into 4 equal-count runs ("rank quartiles"), packed into
    128-edge tiles (edge p of tile g sits on partition p of the gather
    output).
  - Device, per block: FOUR dma_gather calls (one per rank quartile, on
    SWDGE queues 0-3, each against a per-(block,queue) base view of x so
    offsets stay in int16) pull the block's x[src] rows (fp16) from HBM
    into SBUF as [128 edges, T, 128 feat]. Desc-gen for queue q runs on
    Q7 core pair q; the equal-count split keeps all four pairs busy at
    the measured ~8.4ns/descriptor rate, which is the kernel's wall.
    Indices are 0-padded (pad slots gather x[base]; their one-hot rows
    are zero) so every gx slot is written - no stale-SBUF NaN hazard.
  - The scatter one-hot S0 (0/1 structure, built host-side as uint8) is
    streamed from DRAM and scaled on DVE by vs = w[a]*val[e] (computed
    on device).
  - TensorE accumulates S^T @ X into a PSUM block of 128 output rows.
  - Epilogue pass A (per block): Square+row-accum on ScalarE -> ss, raw
    psum copied to SBUF. Pass B runs in 3 chunks (Sqrt + reciprocal over
    the chunk's columns, then exact GELU with per-partition scale), which
    bounds activation-table reloads and overlaps output DMA with the
    remaining blocks.
  - No collectives - host concatenates the 8 per-core row shards.
"""

import sys

sys.path.insert(0, "/opt/trn_rl_repo")

import os
from contextlib import ExitStack

import numpy as np

import concourse.bass as bass
import concourse.tile as tile
from concourse import bacc, library_config, mybir
from concourse.bass_utils import run_bass_kernel_spmd

N_NODES = 50000
N_HID = 128
N_ADJ = 4
N_EDGE = 600000
N_CORES = 8
RPC = N_NODES // N_CORES          # 6250 destination rows per core
BW = 128                          # destination rows per block
NBLK = (RPC + BW - 1) // BW       # 49 blocks (last block 106 rows)
NQ = 4                            # source quarters == SWDGE queues
QW = (N_NODES + NQ - 1) // NQ     # 12500 source rows per quarter (int16-safe)
EPS = 1e-12

fp16 = mybir.dt.float16
fp32 = mybir.dt.float32
i16 = mybir.dt.int16
u8 = mybir.dt.uint8

LAST_RESULTS = None  # BassKernelResults of the most recent run (for test.py)


def _host_prep(x, weight, adj_src, adj_dst, adj_vals):
    """Partition + sort edges per (core, dst-block, src-quarter); build arrays."""
    x = np.ascontiguousarray(np.asarray(x, dtype=np.float32))
    weight = np.asarray(weight, dtype=np.float32).reshape(N_ADJ)
    src_f = np.asarray(adj_src, dtype=np.int64).reshape(-1)
    dst_f = np.asarray(adj_dst, dtype=np.int64).reshape(-1)
    val_f = np.asarray(adj_vals, dtype=np.float32).reshape(-1)
    aid_f = np.repeat(np.arange(N_ADJ, dtype=np.int64), N_EDGE)

    core = dst_f // RPC
    dloc = dst_f - core * RPC
    blk = dloc // BW                # dst block within core (0..NBLK-1)
    slot = dloc - blk * BW          # dst slot within block (0..BW-1)

    # sort by (core, blk, src) and cut each (core, blk) group into NQ
    # equal-count runs ("rank quartiles") - perfectly balanced SWDGE queues.
    # A quartile of a sorted uniform sample spans ~N_NODES/NQ sources, so
    # per-(blk,q) base views keep offsets well inside int16.
    NG = NBLK * NQ                  # (block, quartile) groups per core
    key_cb = core * NBLK + blk
    order = np.lexsort((src_f, key_cb))
    kcb_s = key_cb[order]
    counts_cb = np.bincount(kcb_s, minlength=N_CORES * NBLK)
    starts_cb = np.zeros(N_CORES * NBLK, dtype=np.int64)
    np.cumsum(counts_cb[:-1], out=starts_cb[1:])
    r0 = np.arange(src_f.size, dtype=np.int64) - np.repeat(starts_cb, counts_cb)
    # capacity-quantized cut: desc-gen works in 128-idx chunks, so cut each
    # (core, blk) run at 128-aligned capacities totalling the minimum chunk
    # count ceil(nb_max/128). The odd chunks rotate across queues by block
    # (issue order unchanged) so the four Q7 pairs carry equal long-run
    # loads and the gx/s0 buffer depth smooths the per-block +-1 imbalance.
    nb_max = counts_cb.reshape(N_CORES, NBLK).max(axis=0)         # [NBLK]
    Ttot = np.maximum((nb_max + 127) // 128, NQ)                  # chunks/blk
    sizes = Ttot[:, None] // NQ + (np.arange(NQ)[None, :] < (Ttot[:, None] % NQ))
    rot = (np.arange(NQ)[None, :] + np.arange(NBLK)[:, None]) % NQ
    caps = 128 * np.take_along_axis(sizes, rot, axis=1)           # [NBLK, NQ]
    cum = np.cumsum(caps, axis=1)                                 # [NBLK, NQ]
    blk_s = kcb_s % NBLK
    qt_s = (r0[:, None] >= cum[blk_s][:, :NQ - 1]).sum(axis=1)
    ks = kcb_s * NQ + qt_s          # sorted ascending along `order` already

    counts = np.bincount(ks, minlength=N_CORES * NG)
    cnt = counts.reshape(N_CORES, NBLK, NQ)
    # per-(block, quartile) gather size: the max core's true count rounded
    # to the 16-idx wrap granularity. Descriptors are generated only up to
    # this, not the 128-rounded tile count - the last tile's tail slots are
    # never written and rely on the one-time gx memset for finite contents.
    NM16 = ((np.maximum(cnt.max(axis=0), 1) + 15) // 16 * 16).astype(np.int64)
    Tg = np.maximum((NM16 + 127) // 128, 1)          # [NBLK,NQ] tiles
    offs = np.zeros(NBLK * NQ + 1, dtype=np.int64)
    np.cumsum(Tg.reshape(-1), out=offs[1:])
    NT = int(offs[-1])

    # within-group rank of each (sorted) edge
    starts = np.zeros(N_CORES * NG, dtype=np.int64)
    np.cumsum(counts[:-1], out=starts[1:])
    r = np.arange(src_f.size, dtype=np.int64) - np.repeat(starts, counts)
    p = r & 127
    t = r >> 7
    core_s = ks // NG
    grp_s = ks % NG                 # (blk*NQ + qt)
    g = offs[grp_s] + t             # global tile column

    # common (blk, qt) gather base = min over cores of the group's first src
    src_sorted = src_f[order]
    first_src = np.full(N_CORES * NG, N_NODES, dtype=np.int64)
    np.minimum.at(first_src, ks, src_sorted)
    base = first_src.reshape(N_CORES, NG).min(axis=0)   # [NG]
    base[first_src.reshape(N_CORES, NG).min(axis=0) >= N_NODES] = 0
    src_rel_all = src_sorted - base[grp_s]
    assert src_rel_all.min() >= 0 and src_rel_all.max() < 32768, (
        src_rel_all.min(), src_rel_all.max())

    # 0-padded indices: pad slots gather x[quarter_base]; their s0 rows are
    # 0, so every gx slot is written - no stale-SBUF NaN hazard and no
    # per-core count registers needed.
    idx16 = np.zeros((N_CORES, 128, NT), dtype=np.int16)
    v4 = np.zeros((N_CORES, 128, N_ADJ, NT), dtype=np.float16)
    s0 = np.zeros((N_CORES, 128, NT, BW), dtype=np.uint8)  # one-hot rows

    idx16[core_s, p, g] = src_rel_all.astype(np.int16)
    v4[core_s, p, aid_f[order], g] = val_f[order].astype(np.float16)
    s0[core_s, p, g, slot[order]] = 1

    # dma_gather idx layout per group: [16, T*8] wrap (idx j at [j%16, j//16]),
    # replicated to 128 partitions. Build the whole [128, NT*8] slab.
    idxw = np.zeros((N_CORES, 128, NT * 8), dtype=np.int16)
    for gi in range(NG):
        t0, t1 = int(offs[gi]), int(offs[gi + 1])
        n = (t1 - t0) * 128
        flat = idx16[:, :, t0:t1].transpose(0, 2, 1).reshape(N_CORES, n)  # j order
        wrapped = flat.reshape(N_CORES, n // 16, 16).transpose(0, 2, 1)
        idxw[:, :, t0 * 8:t1 * 8] = np.tile(wrapped, (1, 8, 1))

    x16 = x.astype(np.float16)
    return x16, weight, idxw, v4, s0, Tg, offs, NT, base.reshape(NBLK, NQ), NM16


def _build_program(Tg, offs, NT, base, NM16):
    """Build the single-core bass program (same for all 8 cores)."""
    nc = bacc.Bacc("TRN2", target_bir_lowering=False, debug=False,
                   num_swdge_queues=4)

    x_d = nc.dram_tensor("x16", [N_NODES, N_HID], fp16, kind="ExternalInput")
    w_d = nc.dram_tensor("w", [1, N_ADJ], fp32, kind="ExternalInput")
    idx_d = nc.dram_tensor("idxw", [128, NT * 8], i16, kind="ExternalInput")
    v4_d = nc.dram_tensor("v4", [128, N_ADJ * NT], fp16, kind="ExternalInput")
    s0_d = nc.dram_tensor("s0", [128, NT * BW], u8, kind="ExternalInput")
    out_d = nc.dram_tensor("out", [RPC, N_HID], fp32, kind="ExternalOutput")

    AF = mybir.ActivationFunctionType
    OP = mybir.AluOpType

    with tile.TileContext(nc) as tc, ExitStack() as ctx:
        meta = ctx.enter_context(tc.tile_pool(name="meta", bufs=1))

        with tc.high_priority():
            nc.gpsimd.load_library(library_config.mlp)

        # idx slab in 8 chunks (block-aligned) so early gathers start sooner
        idx_sb = meta.tile([128, NT * 8], i16, tag="idx")
        bchunk = (NBLK + 7) // 8
        for ci in range(8):
            b0 = ci * bchunk
            b1 = min(NBLK, b0 + bchunk)
            if b0 >= b1:
                continue
            c0 = int(offs[b0 * NQ]) * 8
            c1 = int(offs[b1 * NQ]) * 8
            nc.sync.dma_start(out=idx_sb[:, c0:c1], in_=idx_d[:, c0:c1])

        vs_sb = meta.tile([128, NT], fp16, tag="vs")
        ss_sb = meta.tile([BW, NBLK], fp32, tag="ss")
        raw_sb = meta.tile([BW, NBLK * N_HID], fp32, tag="raw")
        ssm_sb = meta.tile([BW, NBLK], fp32, tag="ssm")
        nrm_sb = meta.tile([BW, NBLK], fp32, tag="nrm")
        inv_sb = meta.tile([BW, NBLK], fp32, tag="inv")
        rpool = ctx.enter_context(tc.tile_pool(name="res", bufs=4))

        # vs[p, g] = sum_a w[a] * v4[p, a, g]  (fp16); v4/tmps freed after
        with tc.tile_pool(name="v4tmp", bufs=1) as v4pool, \
             tc.tile_pool(name="wtmp", bufs=1, space="PSUM") as wppool:
            v4_sb = v4pool.tile([128, N_ADJ * NT], fp16, tag="v4")
            nc.sync.dma_start(out=v4_sb[:], in_=v4_d[:])
            # broadcast w[4] to 128 partitions via a K=1 matmul with ones
            w1_sb = v4pool.tile([1, N_ADJ], fp32, tag="w1")
            nc.sync.dma_start(out=w1_sb[:], in_=w_d[:])
            ones_sb = v4pool.tile([1, 128], fp32, tag="ones")
            nc.vector.memset(ones_sb[:], 1.0)
            w_ps = wppool.tile([128, N_ADJ], fp32, space="PSUM", tag="wps")
            nc.tensor.matmul(out=w_ps[:], lhsT=ones_sb[:], rhs=w1_sb[:],
                             start=True, stop=True)
            w_bc = v4pool.tile([128, N_ADJ], fp32, tag="wbc")
            nc.vector.tensor_copy(w_bc[:], w_ps[:])

            tmp0 = v4pool.tile([128, NT], fp16, tag="vs_tmp0")
            nc.vector.tensor_scalar(
                out=tmp0[:], in0=v4_sb[:, 0:NT], scalar1=w_bc[:, 0:1],
                scalar2=None, op0=OP.mult)
            tmp1 = v4pool.tile([128, NT], fp16, tag="vs_tmp1")
            nc.vector.scalar_tensor_tensor(
                out=tmp1[:], in0=v4_sb[:, NT:2 * NT], scalar=w_bc[:, 1:2],
                in1=tmp0[:], op0=OP.mult, op1=OP.add)
            nc.vector.scalar_tensor_tensor(
                out=tmp0[:], in0=v4_sb[:, 2 * NT:3 * NT], scalar=w_bc[:, 2:3],
                in1=tmp1[:], op0=OP.mult, op1=OP.add)
            nc.vector.scalar_tensor_tensor(
                out=vs_sb[:], in0=v4_sb[:, 3 * NT:4 * NT], scalar=w_bc[:, 3:4],
                in1=tmp0[:], op0=OP.mult, op1=OP.add)

        gpool = ctx.enter_context(tc.tile_pool(name="gx", bufs=4))
        s0pool = ctx.enter_context(tc.tile_pool(name="s0", bufs=4))
        spool = ctx.enter_context(tc.tile_pool(name="s", bufs=2))
        ppool = ctx.enter_context(tc.tile_pool(name="psum", bufs=4, space="PSUM"))
        epool = ctx.enter_context(tc.tile_pool(name="epi", bufs=2))

        # fixed-shape gx buffers, memset once: slots beyond each gather's
        # num_idxs (the last tile's tail) are never written and must hold
        # finite data so 0 * value matmul products stay 0
        GB = 4
        Tqmax = [int(Tg[:, qq].max()) for qq in range(NQ)]
        for _ in range(GB):
            for qq in range(NQ):
                g = gpool.tile([128, Tqmax[qq], N_HID], fp16, tag=f"gx{qq}")
                nc.vector.memset(g[:], 0.0)

        for b in range(NBLK):
            goff = int(offs[b * NQ])
            Ts = [int(Tg[b, qq]) for qq in range(NQ)]
            nt_b = sum(Ts)
            gxs = []
            for qq in range(NQ):
                T = Ts[qq]
                off_q = int(offs[b * NQ + qq])
                NM = int(NM16[b, qq])
                gx = gpool.tile([128, Tqmax[qq], N_HID], fp16, tag=f"gx{qq}")
                nc.gpsimd.dma_gather(
                    out_ap=gx[:, :T, :], in_ap=x_d[int(base[b, qq]):, :],
                    idxs_ap=idx_sb[:, off_q * 8:(off_q + T) * 8],
                    num_idxs=NM, num_idxs_reg=NM, elem_size=N_HID,
                    single_packet=False, queue_num=qq)
                gxs.append(gx)

            # stream the block's one-hot structure and scale it by vs
            s0_sb = s0pool.tile([128, nt_b, BW], u8, tag="s0")
            nc.sync.dma_start(
                out=s0_sb[:],
                in_=s0_d[:, goff * BW:(goff + nt_b) * BW]
                    .rearrange("p (t f) -> p t f", t=nt_b))
            S = spool.tile([128, nt_b, BW], fp16, tag="S")
            nc.vector.tensor_tensor(
                out=S[:], in0=s0_sb[:],
                in1=vs_sb[:, goff:goff + nt_b].to_broadcast([128, nt_b, BW]),
                op=OP.mult)

            psum = ppool.tile([BW, N_HID], fp32, space="PSUM", tag="acc")
            t = 0
            for qq in range(NQ):
                for tq in range(Ts[qq]):
                    nc.tensor.matmul(
                        out=psum[:], lhsT=S[:, t, :], rhs=gxs[qq][:, tq, :],
                        start=(t == 0), stop=(t == nt_b - 1))
                    t += 1

            # epilogue pass A: row sum-of-squares + stash raw block
            sq = epool.tile([BW, N_HID], fp32, tag="sq")
            nc.scalar.activation(out=sq[:], in_=psum[:], func=AF.Square,
                                 accum_out=ss_sb[:, b:b + 1])
            nc.vector.tensor_copy(raw_sb[:, b * N_HID:(b + 1) * N_HID], psum[:])

            # epilogue pass B in chunks (keeps ACT-table thrash bounded while
            # letting output DMAs overlap the remaining blocks)
            if b in (15, 31, 45, NBLK - 1):
                c0 = {15: 0, 31: 16, 45: 32, NBLK - 1: 46}[b]
                c1 = b + 1
                nc.vector.tensor_scalar(
                    out=ssm_sb[:, c0:c1], in0=ss_sb[:, c0:c1],
                    scalar1=float(EPS * EPS), scalar2=None, op0=OP.max)
                nc.scalar.sqrt(nrm_sb[:, c0:c1], ssm_sb[:, c0:c1])
                nc.vector.reciprocal(inv_sb[:, c0:c1], nrm_sb[:, c0:c1])
                for bb in range(c0, c1):
                    res = rpool.tile([BW, N_HID], fp32, tag="res")
                    nc.scalar.activation(
                        out=res[:],
                        in_=raw_sb[:, bb * N_HID:(bb + 1) * N_HID],
                        func=AF.Gelu, scale=inv_sb[:, bb:bb + 1])
                    rows = min(BW, RPC - bb * BW)
                    nc.sync.dma_start(out=out_d[bb * BW:bb * BW + rows, :],
                                      in_=res[:rows, :])

    nc.compile()
    return nc


def kernel(x, weight, adj_src, adj_dst, adj_vals, _trace=None):
    global LAST_RESULTS
    x16, w, idxw, v4, s0, Tg, offs, NT, base, NM16 = _host_prep(
        x, weight, adj_src, adj_dst, adj_vals)

    nc = _build_program(Tg, offs, NT, base, NM16)

    in_maps = []
    for c in range(N_CORES):
        in_maps.append({
            "x16": x16,
            "w": w.reshape(1, N_ADJ),
            "idxw": idxw[c],
            "v4": v4[c].reshape(128, N_ADJ * NT),
            "s0": s0[c].reshape(128, NT * BW),
        })

    if _trace is None:
        _trace = bool(int(os.environ.get("GNN_TRACE", "0")))
    res = run_bass_kernel_spmd(nc, in_maps, list(range(N_CORES)), trace=_trace)
    LAST_RESULTS = res

    out = np.concatenate([res.results[c]["out"] for c in range(N_CORES)], axis=0)
    return out.astype(np.float32)


# revision 24
# speedup vs baseline: 1.0518x; 1.0184x over previous
"""GNN message passing (gnn_message_passing) on 8 Trainium2 NeuronCores.

Computation (see reference):
    out = segment_sum over edges of  w[a] * vals[a,e] * x[src[a,e]]  into rows dst[a,e]
    out = gelu_exact(out / max(||out||_2, 1e-12))   (row-wise L2 normalize)

Strategy (node sharding):
  - Each of the 8 cores owns 6250 destination rows, processed in 49 blocks
    of 128 rows. Host sorts each (core, block)'s incident edges by source
    and cuts them into 4 equal-count runs ("rank quartiles"), packed into
    128-edge tiles (edge p of tile g sits on partition p of the gather
    output).
  - Device, per block: FOUR dma_gather calls (one per rank quartile, on
    SWDGE queues 0-3, each against a per-(block,queue) base view of x so
    offsets stay in int16) pull the block's x[src] rows (fp16) from HBM
    into SBUF as [128 edges, T, 128 feat]. Desc-gen for queue q runs on
    Q7 core pair q; the equal-count split keeps all four pairs busy at
    the measured ~8.4ns/descriptor rate, which is the kernel's wall.
    Indices are 0-padded (pad slots gather x[base]; their one-hot rows
    are zero) so every gx slot is written - no stale-SBUF NaN hazard.
  - The scatter one-hot S0 (0/1 structure, built host-side as uint8) is
    streamed from DRAM and scaled on DVE by vs = w[a]*val[e] (computed
    on device).
  - TensorE accumulates S^T @ X into a PSUM block of 128 output rows.
  - Epilogue pass A (per block): Square+row-accum on ScalarE -> ss, raw
    psum copied to SBUF. Pass B runs in 3 chunks (Sqrt + reciprocal over
    the chunk's columns, then exact GELU with per-partition scale), which
    bounds activation-table reloads and overlaps output DMA with the
    remaining blocks.
  - No collectives - host concatenates the 8 per-core row shards.
"""

import sys

sys.path.insert(0, "/opt/trn_rl_repo")

import os
from contextlib import ExitStack

import numpy as np

import concourse.bass as bass
import concourse.tile as tile
from concourse import bacc, library_config, mybir
from concourse.bass_utils import run_bass_kernel_spmd

N_NODES = 50000
N_HID = 128
N_ADJ = 4
N_EDGE = 600000
N_CORES = 8
RPC = N_NODES // N_CORES          # 6250 destination rows per core
BW = 128                          # destination rows per block
NBLK = (RPC + BW - 1) // BW       # 49 blocks (last block 106 rows)
NQ = 4                            # source quarters == SWDGE queues
QW = (N_NODES + NQ - 1) // NQ     # 12500 source rows per quarter (int16-safe)
EPS = 1e-12

fp16 = mybir.dt.float16
fp32 = mybir.dt.float32
i16 = mybir.dt.int16
u8 = mybir.dt.uint8

LAST_RESULTS = None  # BassKernelResults of the most recent run (for test.py)


def _host_prep(x, weight, adj_src, adj_dst, adj_vals):
    """Partition + sort edges per (core, dst-block, src-quarter); build arrays."""
    x = np.ascontiguousarray(np.asarray(x, dtype=np.float32))
    weight = np.asarray(weight, dtype=np.float32).reshape(N_ADJ)
    src_f = np.asarray(adj_src, dtype=np.int64).reshape(-1)
    dst_f = np.asarray(adj_dst, dtype=np.int64).reshape(-1)
    val_f = np.asarray(adj_vals, dtype=np.float32).reshape(-1)
    aid_f = np.repeat(np.arange(N_ADJ, dtype=np.int64), N_EDGE)

    core = dst_f // RPC
    dloc = dst_f - core * RPC
    blk = dloc // BW                # dst block within core (0..NBLK-1)
    slot = dloc - blk * BW          # dst slot within block (0..BW-1)

    # sort by (core, blk, src) and cut each (core, blk) group into NQ
    # equal-count runs ("rank quartiles") - perfectly balanced SWDGE queues.
    # A quartile of a sorted uniform sample spans ~N_NODES/NQ sources, so
    # per-(blk,q) base views keep offsets well inside int16.
    NG = NBLK * NQ                  # (block, quartile) groups per core
    key_cb = core * NBLK + blk
    order = np.lexsort((src_f, key_cb))
    kcb_s = key_cb[order]
    counts_cb = np.bincount(kcb_s, minlength=N_CORES * NBLK)
    starts_cb = np.zeros(N_CORES * NBLK, dtype=np.int64)
    np.cumsum(counts_cb[:-1], out=starts_cb[1:])
    r0 = np.arange(src_f.size, dtype=np.int64) - np.repeat(starts_cb, counts_cb)
    # capacity-quantized cut: desc-gen works in 128-idx chunks, so cut each
    # (core, blk) run at 128-aligned capacities totalling the minimum chunk
    # count ceil(nb_max/128). The odd chunks rotate across queues by block
    # (issue order unchanged) so the four Q7 pairs carry equal long-run
    # loads and the gx/s0 buffer depth smooths the per-block +-1 imbalance.
    nb_max = counts_cb.reshape(N_CORES, NBLK).max(axis=0)         # [NBLK]
    Ttot = np.maximum((nb_max + 127) // 128, NQ)                  # chunks/blk
    sizes = Ttot[:, None] // NQ + (np.arange(NQ)[None, :] < (Ttot[:, None] % NQ))
    rot = (np.arange(NQ)[None, :] + np.arange(NBLK)[:, None]) % NQ
    caps = 128 * np.take_along_axis(sizes, rot, axis=1)           # [NBLK, NQ]
    cum = np.cumsum(caps, axis=1)                                 # [NBLK, NQ]
    blk_s = kcb_s % NBLK
    qt_s = (r0[:, None] >= cum[blk_s][:, :NQ - 1]).sum(axis=1)
    ks = kcb_s * NQ + qt_s          # sorted ascending along `order` already

    counts = np.bincount(ks, minlength=N_CORES * NG)
    cnt = counts.reshape(N_CORES, NBLK, NQ)
    # per-(block, quartile) gather size: the max core's true count rounded
    # to the 16-idx wrap granularity. Descriptors are generated only up to
    # this, not the 128-rounded tile count - the last tile's tail slots are
    # never written and rely on the one-time gx memset for finite contents.
    NM16 = ((np.maximum(cnt.max(axis=0), 1) + 15) // 16 * 16).astype(np.int64)
    Tg = np.maximum((NM16 + 127) // 128, 1)          # [NBLK,NQ] tiles
    offs = np.zeros(NBLK * NQ + 1, dtype=np.int64)
    np.cumsum(Tg.reshape(-1), out=offs[1:])
    NT = int(offs[-1])

    # within-group rank of each (sorted) edge
    starts = np.zeros(N_CORES * NG, dtype=np.int64)
    np.cumsum(counts[:-1], out=starts[1:])
    r = np.arange(src_f.size, dtype=np.int64) - np.repeat(starts, counts)
    p = r & 127
    t = r >> 7
    core_s = ks // NG
    grp_s = ks % NG                 # (blk*NQ + qt)
    g = offs[grp_s] + t             # global tile column

    # common (blk, qt) gather base = min over cores of the group's first src
    src_sorted = src_f[order]
    first_src = np.full(N_CORES * NG, N_NODES, dtype=np.int64)
    np.minimum.at(first_src, ks, src_sorted)
    base = first_src.reshape(N_CORES, NG).min(axis=0)   # [NG]
    base[first_src.reshape(N_CORES, NG).min(axis=0) >= N_NODES] = 0
    src_rel_all = src_sorted - base[grp_s]
    assert src_rel_all.min() >= 0 and src_rel_all.max() < 32768, (
        src_rel_all.min(), src_rel_all.max())

    # 0-padded indices: pad slots gather x[quarter_base]; their s0 rows are
    # 0, so every gx slot is written - no stale-SBUF NaN hazard and no
    # per-core count registers needed.
    idx16 = np.zeros((N_CORES, 128, NT), dtype=np.int16)
    v4 = np.zeros((N_CORES, 128, N_ADJ, NT), dtype=np.float16)
    s0 = np.zeros((N_CORES, 128, NT, BW), dtype=np.uint8)  # one-hot rows

    idx16[core_s, p, g] = src_rel_all.astype(np.int16)
    v4[core_s, p, aid_f[order], g] = val_f[order].astype(np.float16)
    s0[core_s, p, g, slot[order]] = 1

    # dma_gather idx layout per group: [16, T*8] wrap (idx j at [j%16, j//16]),
    # replicated to 128 partitions. Build the whole [128, NT*8] slab.
    idxw = np.zeros((N_CORES, 128, NT * 8), dtype=np.int16)
    for gi in range(NG):
        t0, t1 = int(offs[gi]), int(offs[gi + 1])
        n = (t1 - t0) * 128
        flat = idx16[:, :, t0:t1].transpose(0, 2, 1).reshape(N_CORES, n)  # j order
        wrapped = flat.reshape(N_CORES, n // 16, 16).transpose(0, 2, 1)
        idxw[:, :, t0 * 8:t1 * 8] = np.tile(wrapped, (1, 8, 1))

    x16 = x.astype(np.float16)
    return x16, weight, idxw, v4, s0, Tg, offs, NT, base.reshape(NBLK, NQ), NM16


def _build_program(Tg, offs, NT, base, NM16):
    """Build the single-core bass program (same for all 8 cores)."""
    nc = bacc.Bacc("TRN2", target_bir_lowering=False, debug=False,
                   num_swdge_queues=4)

    x_d = nc.dram_tensor("x16", [N_NODES, N_HID], fp16, kind="ExternalInput")
    w_d = nc.dram_tensor("w", [1, N_ADJ], fp32, kind="ExternalInput")
    idx_d = nc.dram_tensor("idxw", [128, NT * 8], i16, kind="ExternalInput")
    v4_d = nc.dram_tensor("v4", [128, N_ADJ * NT], fp16, kind="ExternalInput")
    s0_d = nc.dram_tensor("s0", [128, NT * BW], u8, kind="ExternalInput")
    out_d = nc.dram_tensor("out", [RPC, N_HID], fp32, kind="ExternalOutput")

    AF = mybir.ActivationFunctionType
    OP = mybir.AluOpType

    with tile.TileContext(nc) as tc, ExitStack() as ctx:
        meta = ctx.enter_context(tc.tile_pool(name="meta", bufs=1))

        with tc.high_priority():
            nc.gpsimd.load_library(library_config.mlp)

        # idx slab in 8 chunks (block-aligned) so early gathers start sooner
        idx_sb = meta.tile([128, NT * 8], i16, tag="idx")
        bchunk = (NBLK + 7) // 8
        for ci in range(8):
            b0 = ci * bchunk
            b1 = min(NBLK, b0 + bchunk)
            if b0 >= b1:
                continue
            c0 = int(offs[b0 * NQ]) * 8
            c1 = int(offs[b1 * NQ]) * 8
            nc.sync.dma_start(out=idx_sb[:, c0:c1], in_=idx_d[:, c0:c1])

        vs_sb = meta.tile([128, NT], fp16, tag="vs")
        ss_sb = meta.tile([BW, NBLK], fp32, tag="ss")
        raw_sb = meta.tile([BW, NBLK * N_HID], fp32, tag="raw")
        ssm_sb = meta.tile([BW, NBLK], fp32, tag="ssm")
        nrm_sb = meta.tile([BW, NBLK], fp32, tag="nrm")
        inv_sb = meta.tile([BW, NBLK], fp32, tag="inv")
        rpool = ctx.enter_context(tc.tile_pool(name="res", bufs=4))

        # vs[p, g] = sum_a w[a] * v4[p, a, g]  (fp16); v4/tmps freed after
        with tc.tile_pool(name="v4tmp", bufs=1) as v4pool, \
             tc.tile_pool(name="wtmp", bufs=1, space="PSUM") as wppool:
            v4_sb = v4pool.tile([128, N_ADJ * NT], fp16, tag="v4")
            nc.sync.dma_start(out=v4_sb[:], in_=v4_d[:])
            # broadcast w[4] to 128 partitions via a K=1 matmul with ones
            w1_sb = v4pool.tile([1, N_ADJ], fp32, tag="w1")
            nc.sync.dma_start(out=w1_sb[:], in_=w_d[:])
            ones_sb = v4pool.tile([1, 128], fp32, tag="ones")
            nc.vector.memset(ones_sb[:], 1.0)
            w_ps = wppool.tile([128, N_ADJ], fp32, space="PSUM", tag="wps")
            nc.tensor.matmul(out=w_ps[:], lhsT=ones_sb[:], rhs=w1_sb[:],
                             start=True, stop=True)
            w_bc = v4pool.tile([128, N_ADJ], fp32, tag="wbc")
            nc.vector.tensor_copy(w_bc[:], w_ps[:])

            tmp0 = v4pool.tile([128, NT], fp16, tag="vs_tmp0")
            nc.vector.tensor_scalar(
                out=tmp0[:], in0=v4_sb[:, 0:NT], scalar1=w_bc[:, 0:1],
                scalar2=None, op0=OP.mult)
            tmp1 = v4pool.tile([128, NT], fp16, tag="vs_tmp1")
            nc.vector.scalar_tensor_tensor(
                out=tmp1[:], in0=v4_sb[:, NT:2 * NT], scalar=w_bc[:, 1:2],
                in1=tmp0[:], op0=OP.mult, op1=OP.add)
            nc.vector.scalar_tensor_tensor(
                out=tmp0[:], in0=v4_sb[:, 2 * NT:3 * NT], scalar=w_bc[:, 2:3],
                in1=tmp1[:], op0=OP.mult, op1=OP.add)
            nc.vector.scalar_tensor_tensor(
                out=vs_sb[:], in0=v4_sb[:, 3 * NT:4 * NT], scalar=w_bc[:, 3:4],
                in1=tmp0[:], op0=OP.mult, op1=OP.add)

        gpool = ctx.enter_context(tc.tile_pool(name="gx", bufs=4))
        s0pool = ctx.enter_context(tc.tile_pool(name="s0", bufs=4))
        spool = ctx.enter_context(tc.tile_pool(name="s", bufs=3))
        ppool = ctx.enter_context(tc.tile_pool(name="psum", bufs=4, space="PSUM"))
        epool = ctx.enter_context(tc.tile_pool(name="epi", bufs=2))

        # fixed-shape gx buffers, memset once: slots beyond each gather's
        # num_idxs (the last tile's tail) are never written and must hold
        # finite data so 0 * value matmul products stay 0
        GB = 4
        Tqmax = [int(Tg[:, qq].max()) for qq in range(NQ)]
        for _ in range(GB):
            for qq in range(NQ):
                g = gpool.tile([128, Tqmax[qq], N_HID], fp16, tag=f"gx{qq}")
                nc.vector.memset(g[:], 0.0)

        for b in range(NBLK):
            goff = int(offs[b * NQ])
            Ts = [int(Tg[b, qq]) for qq in range(NQ)]
            nt_b = sum(Ts)
            gxs = []
            for qq in range(NQ):
                T = Ts[qq]
                off_q = int(offs[b * NQ + qq])
                NM = int(NM16[b, qq])
                gx = gpool.tile([128, Tqmax[qq], N_HID], fp16, tag=f"gx{qq}")
                nc.gpsimd.dma_gather(
                    out_ap=gx[:, :T, :], in_ap=x_d[int(base[b, qq]):, :],
                    idxs_ap=idx_sb[:, off_q * 8:(off_q + T) * 8],
                    num_idxs=NM, num_idxs_reg=NM, elem_size=N_HID,
                    single_packet=False, queue_num=qq)
                gxs.append(gx)

            # stream the block's one-hot structure and scale it by vs
            s0_sb = s0pool.tile([128, nt_b, BW], u8, tag="s0")
            nc.sync.dma_start(
                out=s0_sb[:],
                in_=s0_d[:, goff * BW:(goff + nt_b) * BW]
                    .rearrange("p (t f) -> p t f", t=nt_b))
            S = spool.tile([128, nt_b, BW], fp16, tag="S")
            nc.vector.tensor_tensor(
                out=S[:], in0=s0_sb[:],
                in1=vs_sb[:, goff:goff + nt_b].to_broadcast([128, nt_b, BW]),
                op=OP.mult)

            psum = ppool.tile([BW, N_HID], fp32, space="PSUM", tag="acc")
            t = 0
            for qq in range(NQ):
                for tq in range(Ts[qq]):
                    nc.tensor.matmul(
                        out=psum[:], lhsT=S[:, t, :], rhs=gxs[qq][:, tq, :],
                        start=(t == 0), stop=(t == nt_b - 1))
                    t += 1

            # epilogue pass A: row sum-of-squares + stash raw block
            sq = epool.tile([BW, N_HID], fp32, tag="sq")
            nc.scalar.activation(out=sq[:], in_=psum[:], func=AF.Square,
                                 accum_out=ss_sb[:, b:b + 1])
            nc.vector.tensor_copy(raw_sb[:, b * N_HID:(b + 1) * N_HID], psum[:])

            # epilogue pass B in chunks (keeps ACT-table thrash bounded while
            # letting output DMAs overlap the remaining blocks)
            if b in (15, 31, 45, NBLK - 1):
                c0 = {15: 0, 31: 16, 45: 32, NBLK - 1: 46}[b]
                c1 = b + 1
                nc.vector.tensor_scalar(
                    out=ssm_sb[:, c0:c1], in0=ss_sb[:, c0:c1],
                    scalar1=float(EPS * EPS), scalar2=None, op0=OP.max)
                nc.scalar.sqrt(nrm_sb[:, c0:c1], ssm_sb[:, c0:c1])
                nc.vector.reciprocal(inv_sb[:, c0:c1], nrm_sb[:, c0:c1])
                for bb in range(c0, c1):
                    res = rpool.tile([BW, N_HID], fp32, tag="res")
                    nc.scalar.activation(
                        out=res[:],
                        in_=raw_sb[:, bb * N_HID:(bb + 1) * N_HID],
                        func=AF.Gelu, scale=inv_sb[:, bb:bb + 1])
                    rows = min(BW, RPC - bb * BW)
                    nc.sync.dma_start(out=out_d[bb * BW:bb * BW + rows, :],
                                      in_=res[:rows, :])

    nc.compile()
    return nc


def kernel(x, weight, adj_src, adj_dst, adj_vals, _trace=None):
    global LAST_RESULTS
    x16, w, idxw, v4, s0, Tg, offs, NT, base, NM16 = _host_prep(
        x, weight, adj_src, adj_dst, adj_vals)

    nc = _build_program(Tg, offs, NT, base, NM16)

    in_maps = []
    for c in range(N_CORES):
        in_maps.append({
            "x16": x16,
            "w": w.reshape(1, N_ADJ),
            "idxw": idxw[c],
            "v4": v4[c].reshape(128, N_ADJ * NT),
            "s0": s0[c].reshape(128, NT * BW),
        })

    if _trace is None:
        _trace = bool(int(os.environ.get("GNN_TRACE", "0")))
    res = run_bass_kernel_spmd(nc, in_maps, list(range(N_CORES)), trace=_trace)
    LAST_RESULTS = res

    out = np.concatenate([res.results[c]["out"] for c in range(N_CORES)], axis=0)
    return out.astype(np.float32)
